# revision 40
# baseline (speedup 1.0000x reference)
"""BinaryTreeLSTM on 8 TRN2 NeuronCores (Bass/Tile).

Data-parallel over trees: 32 trees per core. Per core:
  * leaves: dma_gather (num_idxs=1024; >1024 overflows the SWDGE descriptor
    ring and wedges the device) pulls subtoken embeddings bf16 from DRAM,
    round-robined over 4 SWDGE queues (num_swdge_queues=4) -- each gather's
    descriptors are generated by ONE Q7 core-pair selected by queue_num, so
    4 queues give ~4x descriptor throughput (~2.3us/gather effective vs
    ~9.5us single-queue). Keep the gather tile pool deep (bufs=8) or queue
    starvation serializes them again.
  * token sum: one DVE add per gather (subtoken pairs), then the remaining
    sum-of-4 is fused into PE transposes: plain matmuls against identity
    accumulating in f32 PSUM (NOTE: is_transpose=True does NOT accumulate
    correctly on HW even though CoreSim says it does).
  * iou via PE, gates on ACT (fp16 sigmoids: one [4,gc] instr replaces two;
    fp16 keeps f-gate error ~5e-4 where bf16 blows the error budget), h/c on
    DVE; c-state in fp16 (safe), h in bf16.
  * levels d=8..6 per tree sub-group (tapered SUB_SIZES so compute overlaps
    the gather stream); levels d=5..0 run once, merged across all 32 trees,
    so the latency-bound tiny top levels are one dependency chain instead of
    one per sub-group. H-major state [H, nodes]; children of a level are the
    even/odd stride-2 slices of the previous level (works across the merged
    tree blocks since blocks are even-sized).
  * level psum split into two tags (pio: i,o / pfu: fl,fr,u) so the next
    group's matmuls only wait for the matching ACT read. Gate blocks are
    padded to G=512 columns: a matmul destination must not cross a 2KB PSUM
    bank boundary (CoreSim catches this; HW silently corrupts).

Do NOT interleave the emission of one sub-group's level groups with the next
sub-group's leaf instructions: it measured ~10us faster but produced
intermittent wrong results / NaNs on HW (scheduler race not caught by
CoreSim). Sequential emission is stable.

Hardcoded per the problem's input spec: mask is all ones (mean = sum/8, folded
into the ACT input scale), h/c initial states are zeros (leaves get no c_in),
and b_iou/Uf_b are zeros (no biases anywhere).
"""

import sys
from contextlib import ExitStack

import numpy as np
import ml_dtypes

sys.path.insert(0, "/opt/trn_rl_repo")

import concourse.bass as bass
import concourse.tile as tile
from concourse import bacc, mybir
from concourse.masks import make_identity

# problem constants
B, D, H, X, V, L = 256, 9, 128, 128, 30000, 8
N = 2 ** (D + 1) - 1      # 1023 nodes per tree
NCORES = 8
TPC = B // NCORES         # 32 trees per core
SUB_SIZES = [6, 6, 6, 6, 4, 2, 2]  # tapered sub-groups: big first (SBUF sizing)
LPT = 2 ** D              # 512 leaves per tree
GI_PER_TREE = LPT * L     # 4096 gather indices per tree
NG = 4                    # gathers per tree (1024 idxs each), one per queue
GN = GI_PER_TREE // NG    # 1024
G = 512                   # node-group size for the level phase
MERGE_D = 3               # levels d<=MERGE_D run merged across all trees
MN = 2 ** (MERGE_D + 1)   # nodes per tree entering the merged phase (64)

F32 = mybir.dt.float32
BF16 = mybir.dt.bfloat16
I16 = mybir.dt.int16
F16 = mybir.dt.float16
bf16 = ml_dtypes.bfloat16

SIG = mybir.ActivationFunctionType.Sigmoid
TANH = mybir.ActivationFunctionType.Tanh


def level_group(nc, ppool, gates, uiou_t, uf_t, h_prev, c_prev, h_cur, c_cur,
                g0, gc, out_base, root_sink=None):
    """One node-group of one tree level."""
    if True:
        hl = h_prev[:, 2 * g0:2 * (g0 + gc):2]
        hr = h_prev[:, 2 * g0 + 1:2 * (g0 + gc):2]
        # two psum tags so group k+1's matmuls only wait on the matching
        # ACT read of group k (pio: i,o -> 2 banks; pfu: fl,fr,u -> 3 banks)
        piof = ppool.tile([128, 2, G], F32, tag="pio")
        pfuf = ppool.tile([128, 3, G], F32, tag="pfu")
        pio = piof[:, :, 0:gc]
        pfu = pfuf[:, :, 0:gc]
        for blk in range(2):  # i, o
            nc.tensor.matmul(
                pio[:, blk, :], lhsT=uiou_t[:, 0, blk * 128:(blk + 1) * 128],
                rhs=hl, start=True, stop=False)
            nc.tensor.matmul(
                pio[:, blk, :], lhsT=uiou_t[:, 1, blk * 128:(blk + 1) * 128],
                rhs=hr, start=False, stop=True)
        sio = gates.tile([128, 2, gc], F16, tag="lsio")  # sig(i, o)
        nc.scalar.activation(sio[:], pio[:], SIG)
        for blk in range(2):  # fl, fr
            nc.tensor.matmul(
                pfu[:, blk, :], lhsT=uf_t[:, 0, blk * 128:(blk + 1) * 128],
                rhs=hl, start=True, stop=False)
            nc.tensor.matmul(
                pfu[:, blk, :], lhsT=uf_t[:, 1, blk * 128:(blk + 1) * 128],
                rhs=hr, start=False, stop=True)
        nc.tensor.matmul(
            pfu[:, 2, :], lhsT=uiou_t[:, 0, 256:384],
            rhs=hl, start=True, stop=False)
        nc.tensor.matmul(
            pfu[:, 2, :], lhsT=uiou_t[:, 1, 256:384],
            rhs=hr, start=False, stop=True)
        sff = gates.tile([128, 2, gc], F16, tag="lsff")  # sig(fl, fr)
        nc.scalar.activation(sff[:], pfu[:, 0:2, :], SIG)
        tu = gates.tile([128, gc], BF16, tag="ltu")
        nc.scalar.activation(tu[:], pfu[:, 2, :], TANH)

        t1 = gates.tile([128, gc], F16, tag="t1")
        nc.vector.tensor_mul(t1[:], sff[:, 0, :], c_prev[:, 2 * g0:2 * (g0 + gc):2])
        t2 = gates.tile([128, gc], F16, tag="t2")
        nc.vector.tensor_mul(t2[:], sff[:, 1, :], c_prev[:, 2 * g0 + 1:2 * (g0 + gc):2])
        cin = gates.tile([128, gc], F16, tag="cin")
        nc.vector.tensor_add(cin[:], t1[:], t2[:])
        t3 = gates.tile([128, gc], F16, tag="t3")
        nc.vector.tensor_mul(t3[:], sio[:, 0, :], tu[:])
        csl = c_cur[:, out_base + g0:out_base + g0 + gc]
        nc.vector.tensor_add(csl, t3[:], cin[:])
        tch = gates.tile([128, gc], BF16, tag="ltc")
        nc.scalar.activation(tch[:], csl, TANH)
        if root_sink is not None:
            nc.vector.tensor_mul(root_sink[:], sio[:, 1, :], tch[:])
        else:
            nc.vector.tensor_mul(h_cur[:, out_base + g0:out_base + g0 + gc],
                                 sio[:, 1, :], tch[:])


def level_step(nc, ppool, gates, uiou_t, uf_t, h_prev, c_prev, h_cur, c_cur,
               n, out_base, root_sink=None):
    for g0 in range(0, n, G):
        gc = min(G, n - g0)
        level_group(nc, ppool, gates, uiou_t, uf_t, h_prev, c_prev,
                    h_cur, c_cur, g0, gc, out_base, root_sink)


def sub_levels_gen(nc, ppool, gates, state, uiou_t, uf_t, h_leaf, c_leaf,
                   tps, tree_base, h_m, c_m):
    """Generator emitting one level-group per next(); levels d=8..MERGE_D+1
    of one sub-group, the last level writing into the merged h_m/c_m."""
    h_prev, c_prev = h_leaf, c_leaf
    for d in range(D - 1, MERGE_D, -1):
        n = tps * (2 ** d)
        last = d == MERGE_D + 1
        if last:
            h_cur, c_cur = h_m, c_m
            out_base = tree_base * MN
        else:
            h_cur = state.tile([128, n], BF16, tag=f"h_{d % 2}")
            c_cur = state.tile([128, n], F16, tag=f"c_{d % 2}")
            out_base = 0
        for g0 in range(0, n, G):
            gc = min(G, n - g0)
            level_group(nc, ppool, gates, uiou_t, uf_t, h_prev, c_prev,
                        h_cur, c_cur, g0, gc, out_base)
            yield
        if not last:
            h_prev, c_prev = h_cur, c_cur


def pump(gen, k):
    taken = 0
    while gen is not None and taken < k:
        if next(gen, "END") == "END":
            return taken, None
        taken += 1
    return taken, gen


def build_tile_kernel(ctx, tc, emb, idx, wiou, uiou, uf, out):
    nc = tc.nc

    singles = ctx.enter_context(tc.tile_pool(name="singles", bufs=1))
    gpool = ctx.enter_context(tc.tile_pool(name="gather", bufs=8))
    spool = ctx.enter_context(tc.tile_pool(name="sums", bufs=3))
    state = ctx.enter_context(tc.tile_pool(name="state", bufs=1))
    gates = ctx.enter_context(tc.tile_pool(name="gates", bufs=3))
    ppool = ctx.enter_context(tc.tile_pool(name="psum", bufs=1, space="PSUM"))

    # constants
    idx_t = singles.tile([128, TPC * GI_PER_TREE // 16], I16)
    c1 = GI_PER_TREE // 16  # first tree's columns first so gather 0 starts early
    nc.sync.dma_start(out=idx_t[:, 0:c1], in_=idx[:, 0:c1])
    nc.sync.dma_start(out=idx_t[:, c1:12 * c1], in_=idx[:, c1:12 * c1])
    nc.sync.dma_start(out=idx_t[:, 12 * c1:], in_=idx[:, 12 * c1:])
    wiou_t = singles.tile([X, 3 * H], BF16)
    nc.sync.dma_start(out=wiou_t[:], in_=wiou)
    uiou_t = singles.tile([H, 2, 3 * H], BF16)
    nc.sync.dma_start(out=uiou_t[:], in_=uiou)
    uf_t = singles.tile([H, 2, 2 * H], BF16)
    nc.sync.dma_start(out=uf_t[:], in_=uf)
    ident = singles.tile([128, 128], BF16)
    make_identity(nc, ident[:])

    # merged level-6 state, written by every sub-group, read by the merged phase
    h_m = singles.tile([128, TPC * MN], BF16)
    c_m = singles.tile([128, TPC * MN], F16)

    tree_base = 0
    for tps in SUB_SIZES:
        # ---------------- leaves ----------------
        h_leaf = state.tile([128, tps * LPT], BF16, tag="h_leaf")
        c_leaf = state.tile([128, tps * LPT], F16, tag="c_leaf")
        for t in range(tps):
            tree = tree_base + t
            # 4 gathers of 1024 rows; gather i covers subtokens s = 2i, 2i+1
            sis = []
            for i in range(NG):
                c0 = tree * (GI_PER_TREE // 16) + i * (GN // 16)
                gd = gpool.tile([128, 8, 128], BF16, tag="gdst")
                nc.gpsimd.dma_gather(
                    gd[:], emb, idx_t[:, c0:c0 + GN // 16],
                    num_idxs=GN, num_idxs_reg=GN, elem_size=X, transpose=False,
                    queue_num=i)
                si = spool.tile([128, 4, 128], BF16, tag=f"s{i}")
                nc.vector.tensor_add(si[:], gd[:, 0:4, :], gd[:, 4:8, :])
                sis.append(si)
            # transpose-and-sum on PE: ptr[x, a, leaf] = sum_i si_i[:, a, :]^T
            # (regular matmul vs identity: si^T with true f32 psum accumulation)
            ptr = ppool.tile([128, 4, 128], F32, tag="pleaf")
            for a in range(4):
                for i in range(NG):
                    nc.tensor.matmul(
                        ptr[:, a, :], lhsT=sis[i][:, a, :], rhs=ident[:],
                        start=(i == 0), stop=(i == NG - 1))
            xsT = spool.tile([128, 4, 128], BF16, tag="xsT")
            nc.vector.tensor_copy(xsT[:], ptr[:])

            rhs = xsT[:].rearrange("p a b -> p (a b)")  # [128, 512]
            pl = ppool.tile([128, 3, LPT], F32, tag="pleaf")
            for blk in range(3):  # i, o, u
                nc.tensor.matmul(
                    pl[:, blk, :], lhsT=wiou_t[:, blk * 128:(blk + 1) * 128],
                    rhs=rhs, start=True, stop=True)
            # gates; scale=1/8 folds the masked-mean divide into ACT
            sio = gates.tile([128, 2, LPT], BF16, tag="sio")
            nc.scalar.activation(sio[:], pl[:, 0:2, :], SIG, scale=0.125)
            tu = gates.tile([128, LPT], BF16, tag="tu")
            nc.scalar.activation(tu[:], pl[:, 2, :], TANH, scale=0.125)
            csl = c_leaf[:, t * LPT:(t + 1) * LPT]
            nc.vector.tensor_mul(csl, sio[:, 0, :], tu[:])
            tch = gates.tile([128, LPT], BF16, tag="tc")
            nc.scalar.activation(tch[:], csl, TANH)
            nc.vector.tensor_mul(h_leaf[:, t * LPT:(t + 1) * LPT], sio[:, 1, :], tch[:])

        # sequential emission: this sub-group's levels right after its leaves
        own_gen = sub_levels_gen(nc, ppool, gates, state, uiou_t, uf_t,
                                 h_leaf, c_leaf, tps, tree_base, h_m, c_m)
        while own_gen is not None:
            _, own_gen = pump(own_gen, 1 << 30)
        tree_base += tps

    # all sub-group writes into h_m/c_m must land before the merged phase
    tc.strict_bb_all_engine_barrier()

    # ---------------- merged levels d=5..0 over all 32 trees ----------------
    h_prev, c_prev = h_m, c_m
    h_root = None
    for d in range(MERGE_D, -1, -1):
        n = TPC * (2 ** d)
        is_root = d == 0
        h_root = None
        if is_root:
            h_root = singles.tile([128, TPC], F32, name="h_root")
        h_cur = None if is_root else state.tile([128, n], BF16, tag=f"h_{d % 2}")
        c_cur = state.tile([128, n], F16, tag=f"c_{d % 2}")
        level_step(nc, ppool, gates, uiou_t, uf_t, h_prev, c_prev,
                   h_cur, c_cur, n, 0, root_sink=h_root)
        h_prev, c_prev = h_cur, c_cur
    # H-major [H, trees] -> DRAM [trees, H] via transposed AP
    nc.sync.dma_start(out=out[:, :].rearrange("t p -> p t"), in_=h_root[:])


def build_program():
    nc = bacc.Bacc("TRN2", target_bir_lowering=False, debug=False,
                   num_swdge_queues=4)
    emb = nc.dram_tensor("emb", [V, X], BF16, kind="ExternalInput").ap()
    idx = nc.dram_tensor("idx", [128, TPC * GI_PER_TREE // 16], I16,
                         kind="ExternalInput").ap()
    wiou = nc.dram_tensor("wiou", [X, 3 * H], BF16, kind="ExternalInput").ap()
    uiou = nc.dram_tensor("uiou", [H, 2, 3 * H], BF16, kind="ExternalInput").ap()
    uf = nc.dram_tensor("uf", [H, 2, 2 * H], BF16, kind="ExternalInput").ap()
    out = nc.dram_tensor("out", [TPC, H], F32, kind="ExternalOutput").ap()

    with tile.TileContext(nc) as tc:
        with ExitStack() as ctx:
            build_tile_kernel(ctx, tc, emb, idx, wiou, uiou, uf, out)
    nc.compile()
    return nc


def pack_inputs(subtokens, emb, W_iou, U_iou, Uf_W):
    """Host-side packing: shard trees, reorder leaf subtoken indices into the
    dma_gather layout, pre-transpose/cast weights."""
    emb_bf = np.ascontiguousarray(np.asarray(emb, np.float32).astype(bf16))
    wiou_p = np.ascontiguousarray(np.asarray(W_iou, np.float32).astype(bf16))
    uiou_p = np.ascontiguousarray(
        np.asarray(U_iou, np.float32).astype(bf16).reshape(2, H, 3 * H).transpose(1, 0, 2))
    uf_p = np.ascontiguousarray(
        np.asarray(Uf_W, np.float32).astype(bf16).reshape(2, H, 2 * H).transpose(1, 0, 2))

    sub3 = np.asarray(subtokens).reshape(B, N, L)[:, 2 ** D - 1:, :]  # [B, 512, 8]
    in_maps = []
    for cidx in range(NCORES):
        st = sub3[cidx * TPC:(cidx + 1) * TPC]          # [32, 512, 8]
        # gather element g (within a tree) = s*512 + j -> value st[t, j, s]
        A = st.transpose(0, 2, 1).reshape(TPC, GI_PER_TREE)
        # dma_gather reads element g from idxs[g % 16, g // 16]
        A = A.reshape(TPC, GI_PER_TREE // 16, 16).transpose(2, 0, 1)  # [16, t, col]
        A = A.reshape(16, TPC * GI_PER_TREE // 16).astype(np.int16)
        idxs = np.ascontiguousarray(np.tile(A, (8, 1)))  # replicate to 128 partitions
        in_maps.append({
            "emb": emb_bf, "idx": idxs, "wiou": wiou_p, "uiou": uiou_p, "uf": uf_p,
        })
    return in_maps


_NC_CACHE = None


def kernel(subtokens, mask, h, c, emb, W_iou, U_iou, b_iou, Uf_W, Uf_b):
    """Full inputs in, full output out ([256, 128] f32 root hidden states)."""
    global _NC_CACHE
    from concourse.bass_utils import run_bass_kernel_spmd

    if _NC_CACHE is None:
        _NC_CACHE = build_program()
    nc = _NC_CACHE
    in_maps = pack_inputs(subtokens, emb, W_iou, U_iou, Uf_W)
    res = run_bass_kernel_spmd(nc, in_maps, list(range(NCORES)))
    out = np.concatenate([res.results[i]["out"] for i in range(NCORES)], axis=0)
    return np.ascontiguousarray(out.astype(np.float32))


if __name__ == "__main__":
    nc = build_program()
    print("program built ok")


# revision 41
# speedup vs baseline: 1.0082x; 1.0082x over previous
"""BinaryTreeLSTM on 8 TRN2 NeuronCores (Bass/Tile).

Data-parallel over trees: 32 trees per core. Per core:
  * leaves: dma_gather (num_idxs=1024; >1024 overflows the SWDGE descriptor
    ring and wedges the device) pulls subtoken embeddings bf16 from DRAM,
    round-robined over 4 SWDGE queues (num_swdge_queues=4) -- each gather's
    descriptors are generated by ONE Q7 core-pair selected by queue_num, so
    4 queues give ~4x descriptor throughput (~2.3us/gather effective vs
    ~9.5us single-queue). Keep the gather tile pool deep (bufs=8) or queue
    starvation serializes them again.
  * token sum: one DVE add per gather (subtoken pairs), then the remaining
    sum-of-4 is fused into PE transposes: plain matmuls against identity
    accumulating in f32 PSUM (NOTE: is_transpose=True does NOT accumulate
    correctly on HW even though CoreSim says it does).
  * iou via PE, gates on ACT (fp16 sigmoids: one [4,gc] instr replaces two;
    fp16 keeps f-gate error ~5e-4 where bf16 blows the error budget), h/c on
    DVE; c-state in fp16 (safe), h in bf16.
  * levels d=8..6 per tree sub-group (tapered SUB_SIZES so compute overlaps
    the gather stream); levels d=5..0 run once, merged across all 32 trees,
    so the latency-bound tiny top levels are one dependency chain instead of
    one per sub-group. H-major state [H, nodes]; children of a level are the
    even/odd stride-2 slices of the previous level (works across the merged
    tree blocks since blocks are even-sized).
  * level psum split into two tags (pio: i,o / pfu: fl,fr,u) so the next
    group's matmuls only wait for the matching ACT read. Gate blocks are
    padded to G=512 columns: a matmul destination must not cross a 2KB PSUM
    bank boundary (CoreSim catches this; HW silently corrupts).

Do NOT interleave the emission of one sub-group's level groups with the next
sub-group's leaf instructions: it measured ~10us faster but produced
intermittent wrong results / NaNs on HW (scheduler race not caught by
CoreSim). Sequential emission is stable.

Hardcoded per the problem's input spec: mask is all ones (mean = sum/8, folded
into the ACT input scale), h/c initial states are zeros (leaves get no c_in),
and b_iou/Uf_b are zeros (no biases anywhere).
"""

import sys
from contextlib import ExitStack

import numpy as np
import ml_dtypes

sys.path.insert(0, "/opt/trn_rl_repo")

import concourse.bass as bass
import concourse.tile as tile
from concourse import bacc, mybir
from concourse.masks import make_identity

# problem constants
B, D, H, X, V, L = 256, 9, 128, 128, 30000, 8
N = 2 ** (D + 1) - 1      # 1023 nodes per tree
NCORES = 8
TPC = B // NCORES         # 32 trees per core
SUB_SIZES = [6, 6, 6, 6, 4, 2, 2]  # tapered sub-groups: big first (SBUF sizing)
LPT = 2 ** D              # 512 leaves per tree
GI_PER_TREE = LPT * L     # 4096 gather indices per tree
NG = 4                    # gathers per tree (1024 idxs each), one per queue
GN = GI_PER_TREE // NG    # 1024
G = 512                   # node-group size for the level phase
MERGE_D = 4               # levels d<=MERGE_D run merged across all trees
MN = 2 ** (MERGE_D + 1)   # nodes per tree entering the merged phase (64)

F32 = mybir.dt.float32
BF16 = mybir.dt.bfloat16
I16 = mybir.dt.int16
F16 = mybir.dt.float16
bf16 = ml_dtypes.bfloat16

SIG = mybir.ActivationFunctionType.Sigmoid
TANH = mybir.ActivationFunctionType.Tanh


def level_group(nc, ppool, gates, uiou_t, uf_t, h_prev, c_prev, h_cur, c_cur,
                g0, gc, out_base, root_sink=None):
    """One node-group of one tree level."""
    if True:
        hl = h_prev[:, 2 * g0:2 * (g0 + gc):2]
        hr = h_prev[:, 2 * g0 + 1:2 * (g0 + gc):2]
        # two psum tags so group k+1's matmuls only wait on the matching
        # ACT read of group k (pio: i,o -> 2 banks; pfu: fl,fr,u -> 3 banks)
        piof = ppool.tile([128, 2, G], F32, tag="pio")
        pfuf = ppool.tile([128, 3, G], F32, tag="pfu")
        pio = piof[:, :, 0:gc]
        pfu = pfuf[:, :, 0:gc]
        for blk in range(2):  # i, o
            nc.tensor.matmul(
                pio[:, blk, :], lhsT=uiou_t[:, 0, blk * 128:(blk + 1) * 128],
                rhs=hl, start=True, stop=False)
            nc.tensor.matmul(
                pio[:, blk, :], lhsT=uiou_t[:, 1, blk * 128:(blk + 1) * 128],
                rhs=hr, start=False, stop=True)
        sio = gates.tile([128, 2, gc], F16, tag="lsio")  # sig(i, o)
        nc.scalar.activation(sio[:], pio[:], SIG)
        for blk in range(2):  # fl, fr
            nc.tensor.matmul(
                pfu[:, blk, :], lhsT=uf_t[:, 0, blk * 128:(blk + 1) * 128],
                rhs=hl, start=True, stop=False)
            nc.tensor.matmul(
                pfu[:, blk, :], lhsT=uf_t[:, 1, blk * 128:(blk + 1) * 128],
                rhs=hr, start=False, stop=True)
        nc.tensor.matmul(
            pfu[:, 2, :], lhsT=uiou_t[:, 0, 256:384],
            rhs=hl, start=True, stop=False)
        nc.tensor.matmul(
            pfu[:, 2, :], lhsT=uiou_t[:, 1, 256:384],
            rhs=hr, start=False, stop=True)
        sff = gates.tile([128, 2, gc], F16, tag="lsff")  # sig(fl, fr)
        nc.scalar.activation(sff[:], pfu[:, 0:2, :], SIG)
        tu = gates.tile([128, gc], BF16, tag="ltu")
        nc.scalar.activation(tu[:], pfu[:, 2, :], TANH)

        t1 = gates.tile([128, gc], F16, tag="t1")
        nc.vector.tensor_mul(t1[:], sff[:, 0, :], c_prev[:, 2 * g0:2 * (g0 + gc):2])
        t2 = gates.tile([128, gc], F16, tag="t2")
        nc.vector.tensor_mul(t2[:], sff[:, 1, :], c_prev[:, 2 * g0 + 1:2 * (g0 + gc):2])
        cin = gates.tile([128, gc], F16, tag="cin")
        nc.vector.tensor_add(cin[:], t1[:], t2[:])
        t3 = gates.tile([128, gc], F16, tag="t3")
        nc.vector.tensor_mul(t3[:], sio[:, 0, :], tu[:])
        csl = c_cur[:, out_base + g0:out_base + g0 + gc]
        nc.vector.tensor_add(csl, t3[:], cin[:])
        tch = gates.tile([128, gc], BF16, tag="ltc")
        nc.scalar.activation(tch[:], csl, TANH)
        if root_sink is not None:
            nc.vector.tensor_mul(root_sink[:], sio[:, 1, :], tch[:])
        else:
            nc.vector.tensor_mul(h_cur[:, out_base + g0:out_base + g0 + gc],
                                 sio[:, 1, :], tch[:])


def level_step(nc, ppool, gates, uiou_t, uf_t, h_prev, c_prev, h_cur, c_cur,
               n, out_base, root_sink=None):
    for g0 in range(0, n, G):
        gc = min(G, n - g0)
        level_group(nc, ppool, gates, uiou_t, uf_t, h_prev, c_prev,
                    h_cur, c_cur, g0, gc, out_base, root_sink)


def sub_levels_gen(nc, ppool, gates, state, uiou_t, uf_t, h_leaf, c_leaf,
                   tps, tree_base, h_m, c_m):
    """Generator emitting one level-group per next(); levels d=8..MERGE_D+1
    of one sub-group, the last level writing into the merged h_m/c_m."""
    h_prev, c_prev = h_leaf, c_leaf
    for d in range(D - 1, MERGE_D, -1):
        n = tps * (2 ** d)
        last = d == MERGE_D + 1
        if last:
            h_cur, c_cur = h_m, c_m
            out_base = tree_base * MN
        else:
            h_cur = state.tile([128, n], BF16, tag=f"h_{d % 2}")
            c_cur = state.tile([128, n], F16, tag=f"c_{d % 2}")
            out_base = 0
        for g0 in range(0, n, G):
            gc = min(G, n - g0)
            level_group(nc, ppool, gates, uiou_t, uf_t, h_prev, c_prev,
                        h_cur, c_cur, g0, gc, out_base)
            yield
        if not last:
            h_prev, c_prev = h_cur, c_cur


def pump(gen, k):
    taken = 0
    while gen is not None and taken < k:
        if next(gen, "END") == "END":
            return taken, None
        taken += 1
    return taken, gen


def build_tile_kernel(ctx, tc, emb, idx, wiou, uiou, uf, out):
    nc = tc.nc

    singles = ctx.enter_context(tc.tile_pool(name="singles", bufs=1))
    gpool = ctx.enter_context(tc.tile_pool(name="gather", bufs=3))
    spool = ctx.enter_context(tc.tile_pool(name="sums", bufs=3))
    state = ctx.enter_context(tc.tile_pool(name="state", bufs=1))
    gates = ctx.enter_context(tc.tile_pool(name="gates", bufs=3))
    ppool = ctx.enter_context(tc.tile_pool(name="psum", bufs=1, space="PSUM"))

    # constants
    idx_t = singles.tile([128, TPC * GI_PER_TREE // 16], I16)
    c1 = GI_PER_TREE // 16  # first tree's columns first so gather 0 starts early
    nc.sync.dma_start(out=idx_t[:, 0:c1], in_=idx[:, 0:c1])
    nc.sync.dma_start(out=idx_t[:, c1:12 * c1], in_=idx[:, c1:12 * c1])
    nc.sync.dma_start(out=idx_t[:, 12 * c1:], in_=idx[:, 12 * c1:])
    wiou_t = singles.tile([X, 3 * H], BF16)
    nc.sync.dma_start(out=wiou_t[:], in_=wiou)
    uiou_t = singles.tile([H, 2, 3 * H], BF16)
    nc.sync.dma_start(out=uiou_t[:], in_=uiou)
    uf_t = singles.tile([H, 2, 2 * H], BF16)
    nc.sync.dma_start(out=uf_t[:], in_=uf)
    ident = singles.tile([128, 128], BF16)
    make_identity(nc, ident[:])

    # merged level-6 state, written by every sub-group, read by the merged phase
    h_m = singles.tile([128, TPC * MN], BF16)
    c_m = singles.tile([128, TPC * MN], F16)

    tree_base = 0
    for tps in SUB_SIZES:
        # ---------------- leaves ----------------
        h_leaf = state.tile([128, tps * LPT], BF16, tag="h_leaf")
        c_leaf = state.tile([128, tps * LPT], F16, tag="c_leaf")
        for t in range(tps):
            tree = tree_base + t
            # 4 gathers of 1024 rows into one tile; gather i covers
            # subtokens s = 2i, 2i+1
            gall = gpool.tile([128, NG, 8, 128], BF16, tag="gdst")
            for i in range(NG):
                c0 = tree * (GI_PER_TREE // 16) + i * (GN // 16)
                nc.gpsimd.dma_gather(
                    gall[:, i, :, :], emb, idx_t[:, c0:c0 + GN // 16],
                    num_idxs=GN, num_idxs_reg=GN, elem_size=X, transpose=False,
                    queue_num=i)
            # one add folds the subtoken pairs of all 4 gathers
            si = spool.tile([128, NG, 4, 128], BF16, tag="si")
            nc.vector.tensor_add(si[:], gall[:, :, 0:4, :], gall[:, :, 4:8, :])
            # transpose-and-sum on PE: ptr[x, a, leaf] = sum_i si[:, i, a, :]^T
            # (regular matmul vs identity: si^T with true f32 psum accumulation)
            ptr = ppool.tile([128, 4, 128], F32, tag="pleaf")
            for a in range(4):
                for i in range(NG):
                    nc.tensor.matmul(
                        ptr[:, a, :], lhsT=si[:, i, a, :], rhs=ident[:],
                        start=(i == 0), stop=(i == NG - 1))
            xsT = spool.tile([128, 4, 128], BF16, tag="xsT")
            nc.vector.tensor_copy(xsT[:], ptr[:])

            rhs = xsT[:].rearrange("p a b -> p (a b)")  # [128, 512]
            pl = ppool.tile([128, 3, LPT], F32, tag="pleaf")
            for blk in range(3):  # i, o, u
                nc.tensor.matmul(
                    pl[:, blk, :], lhsT=wiou_t[:, blk * 128:(blk + 1) * 128],
                    rhs=rhs, start=True, stop=True)
            # gates; scale=1/8 folds the masked-mean divide into ACT
            sio = gates.tile([128, 2, LPT], BF16, tag="sio")
            nc.scalar.activation(sio[:], pl[:, 0:2, :], SIG, scale=0.125)
            tu = gates.tile([128, LPT], BF16, tag="tu")
            nc.scalar.activation(tu[:], pl[:, 2, :], TANH, scale=0.125)
            csl = c_leaf[:, t * LPT:(t + 1) * LPT]
            nc.vector.tensor_mul(csl, sio[:, 0, :], tu[:])
            tch = gates.tile([128, LPT], BF16, tag="tc")
            nc.scalar.activation(tch[:], csl, TANH)
            nc.vector.tensor_mul(h_leaf[:, t * LPT:(t + 1) * LPT], sio[:, 1, :], tch[:])

        # sequential emission: this sub-group's levels right after its leaves
        own_gen = sub_levels_gen(nc, ppool, gates, state, uiou_t, uf_t,
                                 h_leaf, c_leaf, tps, tree_base, h_m, c_m)
        while own_gen is not None:
            _, own_gen = pump(own_gen, 1 << 30)
        tree_base += tps

    # all sub-group writes into h_m/c_m must land before the merged phase
    tc.strict_bb_all_engine_barrier()

    # ---------------- merged levels d=5..0 over all 32 trees ----------------
    h_prev, c_prev = h_m, c_m
    h_root = None
    for d in range(MERGE_D, -1, -1):
        n = TPC * (2 ** d)
        is_root = d == 0
        h_root = None
        if is_root:
            h_root = singles.tile([128, TPC], F32, name="h_root")
        h_cur = None if is_root else state.tile([128, n], BF16, tag=f"h_{d % 2}")
        c_cur = state.tile([128, n], F16, tag=f"c_{d % 2}")
        level_step(nc, ppool, gates, uiou_t, uf_t, h_prev, c_prev,
                   h_cur, c_cur, n, 0, root_sink=h_root)
        h_prev, c_prev = h_cur, c_cur
    # H-major [H, trees] -> DRAM [trees, H] via transposed AP
    nc.sync.dma_start(out=out[:, :].rearrange("t p -> p t"), in_=h_root[:])


def build_program():
    nc = bacc.Bacc("TRN2", target_bir_lowering=False, debug=False,
                   num_swdge_queues=4)
    emb = nc.dram_tensor("emb", [V, X], BF16, kind="ExternalInput").ap()
    idx = nc.dram_tensor("idx", [128, TPC * GI_PER_TREE // 16], I16,
                         kind="ExternalInput").ap()
    wiou = nc.dram_tensor("wiou", [X, 3 * H], BF16, kind="ExternalInput").ap()
    uiou = nc.dram_tensor("uiou", [H, 2, 3 * H], BF16, kind="ExternalInput").ap()
    uf = nc.dram_tensor("uf", [H, 2, 2 * H], BF16, kind="ExternalInput").ap()
    out = nc.dram_tensor("out", [TPC, H], F32, kind="ExternalOutput").ap()

    with tile.TileContext(nc) as tc:
        with ExitStack() as ctx:
            build_tile_kernel(ctx, tc, emb, idx, wiou, uiou, uf, out)
    nc.compile()
    return nc


def pack_inputs(subtokens, emb, W_iou, U_iou, Uf_W):
    """Host-side packing: shard trees, reorder leaf subtoken indices into the
    dma_gather layout, pre-transpose/cast weights."""
    emb_bf = np.ascontiguousarray(np.asarray(emb, np.float32).astype(bf16))
    wiou_p = np.ascontiguousarray(np.asarray(W_iou, np.float32).astype(bf16))
    uiou_p = np.ascontiguousarray(
        np.asarray(U_iou, np.float32).astype(bf16).reshape(2, H, 3 * H).transpose(1, 0, 2))
    uf_p = np.ascontiguousarray(
        np.asarray(Uf_W, np.float32).astype(bf16).reshape(2, H, 2 * H).transpose(1, 0, 2))

    sub3 = np.asarray(subtokens).reshape(B, N, L)[:, 2 ** D - 1:, :]  # [B, 512, 8]
    in_maps = []
    for cidx in range(NCORES):
        st = sub3[cidx * TPC:(cidx + 1) * TPC]          # [32, 512, 8]
        # gather element g (within a tree) = s*512 + j -> value st[t, j, s]
        A = st.transpose(0, 2, 1).reshape(TPC, GI_PER_TREE)
        # dma_gather reads element g from idxs[g % 16, g // 16]
        A = A.reshape(TPC, GI_PER_TREE // 16, 16).transpose(2, 0, 1)  # [16, t, col]
        A = A.reshape(16, TPC * GI_PER_TREE // 16).astype(np.int16)
        idxs = np.ascontiguousarray(np.tile(A, (8, 1)))  # replicate to 128 partitions
        in_maps.append({
            "emb": emb_bf, "idx": idxs, "wiou": wiou_p, "uiou": uiou_p, "uf": uf_p,
        })
    return in_maps


_NC_CACHE = None


def kernel(subtokens, mask, h, c, emb, W_iou, U_iou, b_iou, Uf_W, Uf_b):
    """Full inputs in, full output out ([256, 128] f32 root hidden states)."""
    global _NC_CACHE
    from concourse.bass_utils import run_bass_kernel_spmd

    if _NC_CACHE is None:
        _NC_CACHE = build_program()
    nc = _NC_CACHE
    in_maps = pack_inputs(subtokens, emb, W_iou, U_iou, Uf_W)
    res = run_bass_kernel_spmd(nc, in_maps, list(range(NCORES)))
    out = np.concatenate([res.results[i]["out"] for i in range(NCORES)], axis=0)
    return np.ascontiguousarray(out.astype(np.float32))


if __name__ == "__main__":
    nc = build_program()
    print("program built ok")
